# revision 36
# baseline (speedup 1.0000x reference)
"""TopK sparse autoencoder forward pass on 8 TRN2 NeuronCores.

Data-parallel over the batch (512 rows/core), single-pass fp16 encode:

  A. encode: acts = relu((x - b_dec) @ W_enc.T) as ONE fp16 matmul pass.
     TRN2 fp16 matmul = exact fp16 operand rounding + fp32 accumulation at
     full bf16 rate.  The resulting ~2.8e-4 sigma act noise swaps the
     top-64 selection vs the fp32 reference only on near-tie rows.
     W_enc streams once in fp16 (38 MB/core); acts are rounded to fp16 by
     the relu and spilled to DRAM in fp16 (25 MB/core, half the v1 fp32
     spill -- the dominant DMA saving).  fp16 rounding is monotone, so it
     cannot swap the top-k order; it only creates ties at the k boundary,
     which are flagged and host-repaired.  Per-512-chunk top-8 candidates
     are extracted from the fp16 relu bounce (one MAX8 per chunk).
  B. topk: exact top-(k+8) of each row from the fp16 candidate array via
     max8/match_replace rounds, split into a late mid-encode prefix pass
     (per tile, staggered over sc, covering ~80-92% of chunks) and a very
     short suffix merge at encode end.  tau = k-th value (an exact fp16
     value).  Flags: (a) chunk top-8 overflow (candidate set provably
     incomplete, >= so fp16 ties at the boundary are also caught),
     (b) boundary gap < 2e-4, which for fp16 values means an exact tie
     v_k == v_{k+1} (adjacent fp16 values near 1.5 differ by >= 4.9e-4):
     the mask would then select k+1 values.  Flagged rows are recomputed
     exactly on the host (batched GEMM).
  C. mask: enc = (acts >= tau) * acts from the fp16 spill (exact,
     self-consistent), fp16, DMA-transposed to [F, B] tiles.
  D. decode: x_hat = enc @ W_dec.T + b_dec, all fp16 operands (better
     precision than v1's bf16), block-pipelined over 2048-wide F blocks
     so W_dec streams exactly once.
     Block 0's acts never round-trip DRAM (written SBUF-resident in A).

Schedule: decode is block-pipelined with W_dec groups and the
mask/transpose chain 2 blocks ahead and spill reloads 3 ahead; all DMA
is issued from the sync engine (cross-engine issue measurably loses to
a single well-ordered queue here), with W_dec loads FIRST in each
block's sync stream so their transfer completes a full block before the
PE consumes them.  The masks run in place on the reloaded spill tiles,
giving the mask->transpose ring three blocks of buffer slack.  The
fp16 spill halves the decode reload traffic, which in the fp32 version
exceeded the PE's per-block work and re-triggered the PE half-clock
power state (HAM) every block; block 0's chain is split in halves so
the PE restarts early after the top-k merge.  Measured: ~626-645 us
fast-clock (vs 803 us for the fp32-spill baseline); the device
alternates between clock states worth ~±70 us run-to-run.
"""

import numpy as np

ACT_DIM = 768
DICT = 24576
BATCH = 4096
NCORES = 8
ROWS = BATCH // NCORES          # 512 rows per core
NT = ROWS // 128                # 4 row-tiles per core
CH = 512                        # stage-1 chunk width (= one encode col-chunk)
NCH = DICT // CH                # 48 chunks
CANDW = NCH * 8                 # 384 candidates per row
NEG = -60000.0                  # match_replace sentinel (fp16-representable)
GAP_DELTA = 2.0e-4              # boundary-tie host-repair threshold
NA = ACT_DIM // 128             # 6 K-chunks

_cache = {}


def _build(k: int, with_benc: bool):
    import concourse.bass as bass
    import concourse.mybir as mybir
    from concourse import bacc
    from concourse import tile

    f32 = mybir.dt.float32
    fp16 = mybir.dt.float16
    ROUNDS = (k + 7) // 8 + 1   # one extra round to expose v_{k+1} (gap flag)

    nc = bacc.Bacc("TRN2", target_bir_lowering=False, debug=False,
                   num_devices=NCORES)

    xr_d = nc.dram_tensor("xr", [ACT_DIM, ROWS], fp16, kind="ExternalInput")
    wr_d = nc.dram_tensor("wencR", [ACT_DIM, DICT], fp16, kind="ExternalInput")
    wdecT_d = nc.dram_tensor("wdecT", [DICT // 1024, 128, 8 * ACT_DIM], fp16,
                             kind="ExternalInput")
    bdec_d = nc.dram_tensor("bdec", [1, ACT_DIM], f32, kind="ExternalInput")
    if with_benc:
        benc_d = nc.dram_tensor("benc", [1, DICT], f32, kind="ExternalInput")
    xhat_d = nc.dram_tensor("xhat", [ROWS, ACT_DIM], f32, kind="ExternalOutput")
    flags_d = nc.dram_tensor("flags", [128, NT], f32, kind="ExternalOutput")
    acts_spill = nc.dram_tensor("acts_spill", [NT, 128, DICT], fp16)

    NSC = DICT // 512           # 48 encode column-chunks
    NBLK = DICT // 2048         # 12 C/D blocks
    NF = DICT // 128            # 192 decoder f-chunks
    NFG = DICT // 1024          # 24 wdec groups (2 per block)

    with tile.TileContext(nc) as tc:
        with tc.tile_pool(name="const", bufs=1) as constp, \
             tc.tile_pool(name="cand", bufs=NT) as candp, \
             tc.tile_pool(name="small", bufs=4 * NT + 4) as smallp, \
             tc.tile_pool(name="actsc", bufs=4) as actscp, \
             tc.tile_pool(name="wdec", bufs=4) as wdecp, \
             tc.tile_pool(name="enct", bufs=16) as enctp, \
             tc.tile_pool(name="outsb", bufs=2) as outp:

            bdec_row = constp.tile([1, ACT_DIM], f32)
            nc.sync.dma_start(bdec_row[:], bdec_d.ap())
            bdec_bc = constp.tile([128, ACT_DIM], f32)
            nc.gpsimd.partition_broadcast(bdec_bc[:], bdec_row[:])
            if with_benc:
                benc_row = constp.tile([1, DICT], f32)
                nc.sync.dma_start(benc_row[:], benc_d.ap())

            flags_sb = constp.tile([128, NT], f32)
            cands = [candp.tile([128, CANDW], fp16, tag="cand", name=f"cand{t}")
                     for t in range(NT)]
            taus = [smallp.tile([128, 1], f32, tag="tau", name=f"tau{t}")
                    for t in range(NT)]
            # tails[t]: [0:8R) prefix top-8R, [8R:) suffix chunk candidates,
            # written adjacently so the final merge needs no copies
            PRE = {t: 36 + 2 * t for t in range(NT)}    # prefix pass sc
            sufw = {t: 8 * (NSC - 1 - PRE[t]) for t in range(NT)}
            tails = [candp.tile([128, 8 * ROUNDS + sufw[t]], fp16,
                                tag="tail", name=f"tail{t}", bufs=NT)
                     for t in range(NT)]

            # ---- shared decode-side helpers (pools live at top level) ----
            acs = {}
            ets = {}
            wds = {}

            def load_wd(fg, half=None):
                # half=0/1 loads one 4-chunk half of the group (used during
                # encode so the 1.57MB group DMA never starves the W_enc
                # chunk stream); half=None loads the whole group at once
                if half is None or half == 0:
                    wd = wdecp.tile([128, 8, ACT_DIM], fp16, tag="wd",
                                    name=f"wd{fg}")
                    wds[fg] = wd
                else:
                    wd = wds[fg]
                hs = [0, 1] if half is None else [half]
                for h in hs:
                    nc.sync.dma_start(
                        wd[:, 4 * h:4 * (h + 1), :]
                        .rearrange("p c a -> p (c a)"),
                        wdecT_d.ap()[fg, :, 4 * h * ACT_DIM:
                                     4 * (h + 1) * ACT_DIM])

            def load_ac(t, blk, pool=None):
                ac = (pool or actscp).tile([128, 2048], fp16, tag="ac",
                                           name=f"ac{t}_{blk}")
                nc.gpsimd.dma_start(
                    ac[:],
                    acts_spill.ap()[t, :, blk * 2048:(blk + 1) * 2048])
                acs[(t, blk)] = ac

            def mask_transpose(t, blk, halves=1):
                # mask IN PLACE on the reloaded spill tile: no separate
                # eb pool, so the mask->transpose ring has the full ac
                # pool depth (3 blocks) of slack instead of one block.
                # halves=2 splits the chain so the PE can start consuming
                # the first 8 f-chunks while the rest is still masking
                # (used on the encode->decode critical path for block 0).
                ac = acs.pop((t, blk))
                et = enctp.tile([128, 16, 128], fp16, tag="enct",
                                name=f"et{t}_{blk}")
                hw = 2048 // halves
                for h in range(halves):
                    sl = slice(h * hw, (h + 1) * hw)
                    nc.vector.scalar_tensor_tensor(
                        ac[:, sl], ac[:, sl], taus[t][:, 0:1], ac[:, sl],
                        op0=mybir.AluOpType.is_ge,
                        op1=mybir.AluOpType.mult)
                    nc.sync.dma_start_transpose(
                        et[:, h * (16 // halves):(h + 1) * (16 // halves), :],
                        ac[:, sl])
                ets[(t, blk)] = et

            def topk_rounds(topv, arr):
                for r in range(ROUNDS):
                    nc.vector.max(topv[:, r * 8:(r + 1) * 8], arr)
                    if r < ROUNDS - 1:
                        nc.vector.match_replace(
                            arr, topv[:, r * 8:(r + 1) * 8], arr, NEG)

            # per-tile prefix split of the candidate array: tile t's
            # prefix top-(8R) is computed mid-encode at sc == PRE[t]
            # (staggered so the ~9us DVE chains drain before encode ends).
            # With CH=512 each sc contributes 8 candidates; the prefix
            # covers (PRE[t] + 1) chunks and its top-8R lands in
            # tails[t][:, 0:8R], where the remaining chunks' candidates
            # are appended, so the final merge is one in-place topk.
            def emit_half_tau(t):
                topk_rounds(tails[t][:, 0:8 * ROUNDS],
                            cands[t][:, 0:(PRE[t] + 1) * 8])

            def emit_tau(t):
                # merge: global top-k is within (prefix top-(8R)) U suffix
                topv = smallp.tile([128, 8 * ROUNDS], fp16, tag="topv",
                                   name=f"topv{t}", bufs=NT)
                topk_rounds(topv, tails[t][:])
                nc.vector.tensor_copy(taus[t][:], topv[:, k - 1:k])
                return topv

            def emit_gap(t, topv):
                # tie detector: for fp16 candidate values a boundary gap
                # below GAP_DELTA means v_k == v_{k+1} exactly -- the mask
                # would select k+1 features (or the fp16 rounding collapsed
                # a near-tie the fp32 reference resolves differently)
                gap = smallp.tile([128, 1], f32, tag="gap", name=f"gap{t}",
                                  bufs=NT)
                nc.vector.tensor_tensor(gap[:], topv[:, k - 1:k],
                                        topv[:, k:k + 1],
                                        op=mybir.AluOpType.subtract)
                nc.vector.tensor_scalar(gap[:], gap[:], GAP_DELTA, None,
                                        op0=mybir.AluOpType.is_lt)
                return gap

            # first wdec group early; rest staggered into the encode loop

            # ---------------- Phase A: encode + spill + stage-1 ----------
            with tc.tile_pool(name="xt", bufs=1) as xtp, \
                 tc.tile_pool(name="wenc", bufs=5) as wencp, \
                 tc.tile_pool(name="bounce", bufs=4) as bouncep, \
                 tc.tile_pool(name="encpsum", bufs=6, space="PSUM") as encpsp, \
                 tc.tile_pool(name="bencbc", bufs=2) as bencbcp:

                xr_sb = xtp.tile([128, NA, ROWS], fp16)
                for a in range(NA):
                    nc.sync.dma_start(
                        xr_sb[:, a, :],
                        xr_d.ap()[a * 128:(a + 1) * 128, :])

                c8s = {}
                for sc in range(NSC):
                    if sc in (2, 10, 18, 26):
                        load_wd(sc // 8, half=0)
                    elif sc in (6, 14, 22, 30):
                        load_wd((sc - 4) // 8, half=1)
                    wch = wencp.tile([128, NA, 512], fp16, tag="w",
                                     name=f"w{sc}")
                    nc.sync.dma_start(
                        wch[:],
                        wr_d.ap()[:, sc * 512:(sc + 1) * 512]
                        .rearrange("(a p) c -> p a c", p=128))
                    if with_benc:
                        bb = bencbcp.tile([128, 512], f32, tag="bb")
                        nc.gpsimd.partition_broadcast(
                            bb[:], benc_row[0:1, sc * 512:(sc + 1) * 512])
                    for t in range(NT):
                        ps = encpsp.tile([128, 512], f32, tag="eps")
                        rt = slice(t * 128, (t + 1) * 128)
                        for a in range(NA):
                            nc.tensor.matmul(
                                ps[:], xr_sb[:, a, rt], wch[:, a, :],
                                start=(a == 0), stop=(a == NA - 1))
                        if sc < 4:
                            if (t, 0) not in acs:
                                acs[(t, 0)] = actscp.tile(
                                    [128, 2048], fp16, tag="ac",
                                    name=f"ac{t}_0")
                            bo = acs[(t, 0)][:, sc * 512:(sc + 1) * 512]
                        else:
                            bo = bouncep.tile([128, 512], fp16, tag="bo")
                        if with_benc:
                            nc.vector.tensor_tensor(bo[:], ps[:], bb[:],
                                                    op=mybir.AluOpType.add)
                            nc.scalar.activation(
                                bo[:], bo[:], mybir.ActivationFunctionType.Relu)
                        else:
                            nc.scalar.activation(
                                bo[:], ps[:], mybir.ActivationFunctionType.Relu)
                        if sc >= 4:
                            nc.sync.dma_start(
                                acts_spill.ap()[t, :, sc * 512:(sc + 1) * 512],
                                bo[:])
                        if sc <= PRE[t]:
                            nc.vector.max(cands[t][:, sc * 8:(sc + 1) * 8],
                                          bo[:])
                        else:
                            o = 8 * ROUNDS + (sc - PRE[t] - 1) * 8
                            nc.vector.max(tails[t][:, o:o + 8], bo[:])
                        # prefix pass for this tile: extract the chunk-
                        # overflow stat and the prefix top-(8R) while the
                        # PE finishes the remaining chunks
                        if sc == PRE[t]:
                            c8a = smallp.tile([128, 1], f32, tag="c8a",
                                              name=f"c8a_{t}", bufs=NT)
                            cand3 = cands[t][:].rearrange(
                                "p (c e) -> p c e", e=8)
                            nc.vector.tensor_reduce(
                                c8a[:], cand3[:, 0:PRE[t] + 1, 7:8],
                                axis=mybir.AxisListType.XY,
                                op=mybir.AluOpType.max)
                            c8s[t] = c8a
                            emit_half_tau(t)
                        if sc == NSC - 1:
                            # suffix chunk-overflow stat BEFORE the merge
                            # (the merge's match_replace destroys tails)
                            c8 = smallp.tile([128, 1], f32, tag="c8",
                                             name=f"c8_{t}")
                            tail3 = tails[t][:, 8 * ROUNDS:].rearrange(
                                "p (c e) -> p c e", e=8)
                            nc.vector.tensor_reduce(
                                c8[:], tail3[:, :, 7:8],
                                axis=mybir.AxisListType.XY,
                                op=mybir.AluOpType.max)
                            nc.vector.tensor_tensor(
                                c8[:], c8[:], c8s[t][:],
                                op=mybir.AluOpType.max)
                            # tau next, then immediately unblock the PE
                            # with block 0's mask+transpose; flag math
                            # runs off the critical path afterwards
                            topv = emit_tau(t)
                            mask_transpose(t, 0, halves=2)
                            gap = emit_gap(t, topv)
                            # >= : an fp16 tie of the chunk 8th candidate
                            # with tau can hide an unextracted equal value
                            nc.vector.tensor_tensor(
                                flags_sb[:, t:t + 1], c8[:], taus[t][:],
                                op=mybir.AluOpType.is_ge)
                            nc.vector.tensor_tensor(
                                flags_sb[:, t:t + 1], flags_sb[:, t:t + 1],
                                gap[:], op=mybir.AluOpType.add)

            # -------- Phases C+D: mask/transpose + decode ---------------
            with tc.tile_pool(name="decpsum", bufs=NT, space="PSUM") as decpsp, \
                 tc.tile_pool(name="actsc2", bufs=11) as acp2:
                for t in range(NT):
                    load_ac(t, 1, acp2)
                for t in range(NT):
                    load_ac(t, 2, acp2)
                for t in range(NT):
                    mask_transpose(t, 1)

                pss = [decpsp.tile([128, ACT_DIM], f32, tag="dps",
                                   name=f"dps{t}") for t in range(NT)]
                for blk in range(NBLK):
                    # stay >= 2 blocks ahead with mask/transpose, 3 with
                    # spill reloads.  W_dec loads go FIRST in the sync
                    # stream: at the tail they issue ~14us into the block
                    # and their ~11us transfer lands exactly when the
                    # consuming block wants them (a 5-10us PE stall).
                    for fg in (2 * blk + 4, 2 * blk + 5):
                        if fg < NFG:
                            load_wd(fg)
                    if blk + 2 < NBLK:
                        for t in range(NT):
                            mask_transpose(t, blk + 2)
                    if blk + 3 < NBLK:
                        for t in range(NT):
                            load_ac(t, blk + 3, acp2)
                    wd2 = [wds.pop(2 * blk), wds.pop(2 * blk + 1)]
                    for t in range(NT):
                        for g in range(2):
                            wd = wd2[g]
                            for j in range(8):
                                f = blk * 16 + g * 8 + j
                                lhsT = ets[(t, blk)][:, g * 8 + j, :]
                                st = (f == 0)
                                sp = (f == NF - 1)
                                nc.tensor.matmul(
                                    pss[t][:, 0:512], lhsT, wd[:, j, 0:512],
                                    start=st, stop=sp)
                                nc.tensor.matmul(
                                    pss[t][:, 512:ACT_DIM], lhsT,
                                    wd[:, j, 512:ACT_DIM],
                                    start=st, stop=sp)
                        if blk == NBLK - 1:
                            ot = outp.tile([128, ACT_DIM], f32, tag="ot",
                                           name=f"ot{t}")
                            nc.vector.tensor_tensor(
                                ot[:], pss[t][:], bdec_bc[:],
                                op=mybir.AluOpType.add)
                            nc.sync.dma_start(
                                xhat_d.ap()[t * 128:(t + 1) * 128, :], ot[:])
                    for t in range(NT):
                        ets.pop((t, blk))
                nc.sync.dma_start(flags_d.ap(), flags_sb[:])

    nc.compile()
    return nc


def _get_program(k: int, with_benc: bool):
    key = (k, with_benc)
    if key not in _cache:
        _cache[key] = _build(k, with_benc)
    return _cache[key]


def _host_repair(out, rows, x, W_enc, b_enc, W_dec, b_dec, k):
    rows = np.asarray(rows, dtype=np.int64)
    pre = (x[rows] - b_dec) @ W_enc.T + b_enc          # [R, F]
    acts = np.maximum(pre, 0.0)
    idx = np.argsort(-acts, axis=1, kind="stable")[:, :k]
    enc = np.zeros_like(acts)
    np.put_along_axis(enc, idx, np.take_along_axis(acts, idx, 1), 1)
    out[rows] = enc @ W_dec.T + b_dec


def run(inputs, trace=False):
    from concourse.bass_utils import run_bass_kernel_spmd

    x = np.asarray(inputs["x"], dtype=np.float32)
    W_enc = np.asarray(inputs["W_enc"], dtype=np.float32)
    b_enc = np.asarray(inputs["b_enc"], dtype=np.float32)
    W_dec = np.asarray(inputs["W_dec"], dtype=np.float32)
    b_dec = np.asarray(inputs["b_dec"], dtype=np.float32)
    k = int(np.asarray(inputs["k"]))
    assert x.shape == (BATCH, ACT_DIM) and W_enc.shape == (DICT, ACT_DIM)
    assert 1 <= k <= CANDW - 8

    with_benc = bool(np.any(b_enc))
    nc = _get_program(k, with_benc)

    xT = np.ascontiguousarray((x - b_dec).T, dtype=np.float32).astype(np.float16)
    wencT = np.ascontiguousarray(W_enc.T, dtype=np.float32).astype(np.float16)
    wdecT = np.ascontiguousarray(W_dec.T).astype(np.float16)
    wdec_r = np.ascontiguousarray(
        wdecT.reshape(DICT // 1024, 8, 128, ACT_DIM).transpose(0, 2, 1, 3)
        .reshape(DICT // 1024, 128, 8 * ACT_DIM))
    bdec_row = np.ascontiguousarray(b_dec.reshape(1, ACT_DIM))

    in_maps = []
    for c in range(NCORES):
        sl = slice(c * ROWS, (c + 1) * ROWS)
        m = {
            "xr": np.ascontiguousarray(xT[:, sl]),
            "wencR": wencT,
            "wdecT": wdec_r,
            "bdec": bdec_row,
        }
        if with_benc:
            m["benc"] = np.ascontiguousarray(b_enc.reshape(1, DICT))
        in_maps.append(m)

    res = run_bass_kernel_spmd(nc, in_maps, core_ids=list(range(NCORES)),
                               trace=trace)

    out = np.empty((BATCH, ACT_DIM), dtype=np.float32)
    flagged = []
    for c in range(NCORES):
        out[c * ROWS:(c + 1) * ROWS] = res.results[c]["xhat"]
        fl = res.results[c]["flags"]          # [128, NT]
        for t in range(NT):
            for p in np.nonzero(fl[:, t] > 0)[0]:
                flagged.append(c * ROWS + t * 128 + int(p))
    if flagged:
        _host_repair(out, flagged, x, W_enc, b_enc, W_dec, b_dec, k)
    return out, res, flagged


def kernel(**inputs) -> np.ndarray:
    out, _, _ = run(inputs)
    return out


# revision 37
# speedup vs baseline: 1.2519x; 1.2519x over previous
"""TopK sparse autoencoder forward pass on 8 TRN2 NeuronCores.

Data-parallel over the batch (512 rows/core), single-pass fp16 encode:

  A. encode: acts = relu((x - b_dec) @ W_enc.T) as ONE fp16 matmul pass.
     TRN2 fp16 matmul = exact fp16 operand rounding + fp32 accumulation at
     full bf16 rate.  The resulting ~2.8e-4 sigma act noise swaps the
     top-64 selection vs the fp32 reference only on near-tie rows.
     W_enc streams once in fp16 (38 MB/core); acts are rounded to fp16 by
     the relu and spilled to DRAM in fp16 (25 MB/core, half the v1 fp32
     spill -- the dominant DMA saving).  fp16 rounding is monotone, so it
     cannot swap the top-k order; it only creates ties at the k boundary,
     which are flagged and host-repaired.  Per-512-chunk top-8 candidates
     are extracted from the fp16 relu bounce (one MAX8 per chunk).
  B. topk: exact top-(k+8) of each row from the fp16 candidate array via
     max8/match_replace rounds, split into a late mid-encode prefix pass
     (per tile, staggered over sc, covering ~80-92% of chunks) and a very
     short suffix merge at encode end.  tau = k-th value (an exact fp16
     value).  Flags: (a) chunk top-8 overflow (candidate set provably
     incomplete, >= so fp16 ties at the boundary are also caught),
     (b) boundary gap < 2e-4, which for fp16 values means an exact tie
     v_k == v_{k+1} (adjacent fp16 values near 1.5 differ by >= 4.9e-4):
     the mask would then select k+1 values.  Flagged rows are recomputed
     exactly on the host (batched GEMM).
  C. mask: enc = (acts >= tau) * acts from the fp16 spill (exact,
     self-consistent), fp16, DMA-transposed to [F, B] tiles.
  D. decode: x_hat = enc @ W_dec.T + b_dec, all fp16 operands (better
     precision than v1's bf16), block-pipelined over 2048-wide F blocks
     so W_dec streams exactly once.
     Block 0's acts never round-trip DRAM (written SBUF-resident in A).

Schedule: decode is block-pipelined with W_dec groups and the
mask/transpose chain 2 blocks ahead and spill reloads 3 ahead; all DMA
is issued from the sync engine (cross-engine issue measurably loses to
a single well-ordered queue here), with W_dec loads FIRST in each
block's sync stream so their transfer completes a full block before the
PE consumes them.  The masks run in place on the reloaded spill tiles,
giving the mask->transpose ring three blocks of buffer slack.  The
fp16 spill halves the decode reload traffic, which in the fp32 version
exceeded the PE's per-block work and re-triggered the PE half-clock
power state (HAM) every block; block 0's chain is split in halves so
the PE restarts early after the top-k merge.  Measured: ~626-645 us
fast-clock (vs 803 us for the fp32-spill baseline); the device
alternates between clock states worth ~±70 us run-to-run.
"""

import numpy as np

ACT_DIM = 768
DICT = 24576
BATCH = 4096
NCORES = 8
ROWS = BATCH // NCORES          # 512 rows per core
NT = ROWS // 128                # 4 row-tiles per core
CH = 512                        # stage-1 chunk width (= one encode col-chunk)
NCH = DICT // CH                # 48 chunks
CANDW = NCH * 8                 # 384 candidates per row
NEG = -60000.0                  # match_replace sentinel (fp16-representable)
GAP_DELTA = 2.0e-4              # boundary-tie host-repair threshold
NA = ACT_DIM // 128             # 6 K-chunks

_cache = {}


def _build(k: int, with_benc: bool):
    import concourse.bass as bass
    import concourse.mybir as mybir
    from concourse import bacc
    from concourse import tile

    f32 = mybir.dt.float32
    fp16 = mybir.dt.float16
    ROUNDS = (k + 7) // 8 + 1   # one extra round to expose v_{k+1} (gap flag)

    nc = bacc.Bacc("TRN2", target_bir_lowering=False, debug=False,
                   num_devices=NCORES)

    xr_d = nc.dram_tensor("xr", [ACT_DIM, ROWS], fp16, kind="ExternalInput")
    wr_d = nc.dram_tensor("wencR", [ACT_DIM, DICT], fp16, kind="ExternalInput")
    wdecT_d = nc.dram_tensor("wdecT", [DICT // 1024, 128, 8 * ACT_DIM], fp16,
                             kind="ExternalInput")
    bdec_d = nc.dram_tensor("bdec", [1, ACT_DIM], f32, kind="ExternalInput")
    if with_benc:
        benc_d = nc.dram_tensor("benc", [1, DICT], f32, kind="ExternalInput")
    xhat_d = nc.dram_tensor("xhat", [ROWS, ACT_DIM], f32, kind="ExternalOutput")
    flags_d = nc.dram_tensor("flags", [128, NT], f32, kind="ExternalOutput")
    acts_spill = nc.dram_tensor("acts_spill", [NT, 128, DICT], fp16)

    NSC = DICT // 512           # 48 encode column-chunks
    NBLK = DICT // 2048         # 12 C/D blocks
    NF = DICT // 128            # 192 decoder f-chunks
    NFG = DICT // 1024          # 24 wdec groups (2 per block)

    with tile.TileContext(nc) as tc:
        with tc.tile_pool(name="const", bufs=1) as constp, \
             tc.tile_pool(name="cand", bufs=NT) as candp, \
             tc.tile_pool(name="small", bufs=4 * NT + 4) as smallp, \
             tc.tile_pool(name="actsc", bufs=4) as actscp, \
             tc.tile_pool(name="wdec", bufs=4) as wdecp, \
             tc.tile_pool(name="enct", bufs=16) as enctp, \
             tc.tile_pool(name="outsb", bufs=2) as outp:

            bdec_row = constp.tile([1, ACT_DIM], f32)
            nc.sync.dma_start(bdec_row[:], bdec_d.ap())
            bdec_bc = constp.tile([128, ACT_DIM], f32)
            nc.gpsimd.partition_broadcast(bdec_bc[:], bdec_row[:])
            if with_benc:
                benc_row = constp.tile([1, DICT], f32)
                nc.sync.dma_start(benc_row[:], benc_d.ap())

            flags_sb = constp.tile([128, NT], f32)
            cands = [candp.tile([128, CANDW], fp16, tag="cand", name=f"cand{t}")
                     for t in range(NT)]
            taus = [smallp.tile([128, 1], f32, tag="tau", name=f"tau{t}")
                    for t in range(NT)]
            # tails[t]: [0:8R) prefix top-8R, [8R:) suffix chunk candidates,
            # written adjacently so the final merge needs no copies
            PRE = {t: 36 + 2 * t for t in range(NT)}    # prefix pass sc
            sufw = {t: 8 * (NSC - 1 - PRE[t]) for t in range(NT)}
            tails = [candp.tile([128, 8 * ROUNDS + sufw[t]], fp16,
                                tag="tail", name=f"tail{t}", bufs=NT)
                     for t in range(NT)]

            # ---- shared decode-side helpers (pools live at top level) ----
            acs = {}
            ets = {}
            wds = {}

            def load_wd(fg, half=None):
                # half=0/1 loads one 4-chunk half of the group (used during
                # encode so the 1.57MB group DMA never starves the W_enc
                # chunk stream); half=None loads the whole group at once
                if half is None or half == 0:
                    wd = wdecp.tile([128, 8, ACT_DIM], fp16, tag="wd",
                                    name=f"wd{fg}")
                    wds[fg] = wd
                else:
                    wd = wds[fg]
                hs = [0, 1] if half is None else [half]
                for h in hs:
                    nc.sync.dma_start(
                        wd[:, 4 * h:4 * (h + 1), :]
                        .rearrange("p c a -> p (c a)"),
                        wdecT_d.ap()[fg, :, 4 * h * ACT_DIM:
                                     4 * (h + 1) * ACT_DIM])

            def load_ac(t, blk, pool=None):
                ac = (pool or actscp).tile([128, 2048], fp16, tag="ac",
                                           name=f"ac{t}_{blk}")
                nc.sync.dma_start(
                    ac[:],
                    acts_spill.ap()[t, :, blk * 2048:(blk + 1) * 2048])
                acs[(t, blk)] = ac

            def mask_transpose(t, blk, halves=1):
                # mask IN PLACE on the reloaded spill tile: no separate
                # eb pool, so the mask->transpose ring has the full ac
                # pool depth (3 blocks) of slack instead of one block.
                # halves=2 splits the chain so the PE can start consuming
                # the first 8 f-chunks while the rest is still masking
                # (used on the encode->decode critical path for block 0).
                ac = acs.pop((t, blk))
                et = enctp.tile([128, 16, 128], fp16, tag="enct",
                                name=f"et{t}_{blk}")
                hw = 2048 // halves
                for h in range(halves):
                    sl = slice(h * hw, (h + 1) * hw)
                    nc.vector.scalar_tensor_tensor(
                        ac[:, sl], ac[:, sl], taus[t][:, 0:1], ac[:, sl],
                        op0=mybir.AluOpType.is_ge,
                        op1=mybir.AluOpType.mult)
                    nc.sync.dma_start_transpose(
                        et[:, h * (16 // halves):(h + 1) * (16 // halves), :],
                        ac[:, sl])
                ets[(t, blk)] = et

            def topk_rounds(topv, arr):
                for r in range(ROUNDS):
                    nc.vector.max(topv[:, r * 8:(r + 1) * 8], arr)
                    if r < ROUNDS - 1:
                        nc.vector.match_replace(
                            arr, topv[:, r * 8:(r + 1) * 8], arr, NEG)

            # per-tile prefix split of the candidate array: tile t's
            # prefix top-(8R) is computed mid-encode at sc == PRE[t]
            # (staggered so the ~9us DVE chains drain before encode ends).
            # With CH=512 each sc contributes 8 candidates; the prefix
            # covers (PRE[t] + 1) chunks and its top-8R lands in
            # tails[t][:, 0:8R], where the remaining chunks' candidates
            # are appended, so the final merge is one in-place topk.
            def emit_half_tau(t):
                topk_rounds(tails[t][:, 0:8 * ROUNDS],
                            cands[t][:, 0:(PRE[t] + 1) * 8])

            def emit_tau(t):
                # merge: global top-k is within (prefix top-(8R)) U suffix
                topv = smallp.tile([128, 8 * ROUNDS], fp16, tag="topv",
                                   name=f"topv{t}", bufs=NT)
                topk_rounds(topv, tails[t][:])
                nc.vector.tensor_copy(taus[t][:], topv[:, k - 1:k])
                return topv

            def emit_gap(t, topv):
                # tie detector: for fp16 candidate values a boundary gap
                # below GAP_DELTA means v_k == v_{k+1} exactly -- the mask
                # would select k+1 features (or the fp16 rounding collapsed
                # a near-tie the fp32 reference resolves differently)
                gap = smallp.tile([128, 1], f32, tag="gap", name=f"gap{t}",
                                  bufs=NT)
                nc.vector.tensor_tensor(gap[:], topv[:, k - 1:k],
                                        topv[:, k:k + 1],
                                        op=mybir.AluOpType.subtract)
                nc.vector.tensor_scalar(gap[:], gap[:], GAP_DELTA, None,
                                        op0=mybir.AluOpType.is_lt)
                return gap

            # first wdec group early; rest staggered into the encode loop

            # ---------------- Phase A: encode + spill + stage-1 ----------
            with tc.tile_pool(name="xt", bufs=1) as xtp, \
                 tc.tile_pool(name="wenc", bufs=5) as wencp, \
                 tc.tile_pool(name="bounce", bufs=4) as bouncep, \
                 tc.tile_pool(name="encpsum", bufs=6, space="PSUM") as encpsp, \
                 tc.tile_pool(name="bencbc", bufs=2) as bencbcp:

                xr_sb = xtp.tile([128, NA, ROWS], fp16)
                for a in range(NA):
                    nc.sync.dma_start(
                        xr_sb[:, a, :],
                        xr_d.ap()[a * 128:(a + 1) * 128, :])

                c8s = {}
                for sc in range(NSC):
                    if sc in (2, 10, 18, 26):
                        load_wd(sc // 8, half=0)
                    elif sc in (6, 14, 22, 30):
                        load_wd((sc - 4) // 8, half=1)
                    wch = wencp.tile([128, NA, 512], fp16, tag="w",
                                     name=f"w{sc}")
                    nc.sync.dma_start(
                        wch[:],
                        wr_d.ap()[:, sc * 512:(sc + 1) * 512]
                        .rearrange("(a p) c -> p a c", p=128))
                    if with_benc:
                        bb = bencbcp.tile([128, 512], f32, tag="bb")
                        nc.gpsimd.partition_broadcast(
                            bb[:], benc_row[0:1, sc * 512:(sc + 1) * 512])
                    for t in range(NT):
                        ps = encpsp.tile([128, 512], f32, tag="eps")
                        rt = slice(t * 128, (t + 1) * 128)
                        for a in range(NA):
                            nc.tensor.matmul(
                                ps[:], xr_sb[:, a, rt], wch[:, a, :],
                                start=(a == 0), stop=(a == NA - 1))
                        if sc < 4:
                            if (t, 0) not in acs:
                                acs[(t, 0)] = actscp.tile(
                                    [128, 2048], fp16, tag="ac",
                                    name=f"ac{t}_0")
                            bo = acs[(t, 0)][:, sc * 512:(sc + 1) * 512]
                        else:
                            bo = bouncep.tile([128, 512], fp16, tag="bo")
                        if with_benc:
                            nc.vector.tensor_tensor(bo[:], ps[:], bb[:],
                                                    op=mybir.AluOpType.add)
                            nc.scalar.activation(
                                bo[:], bo[:], mybir.ActivationFunctionType.Relu)
                        else:
                            nc.scalar.activation(
                                bo[:], ps[:], mybir.ActivationFunctionType.Relu)
                        if sc >= 4:
                            nc.sync.dma_start(
                                acts_spill.ap()[t, :, sc * 512:(sc + 1) * 512],
                                bo[:])
                        if sc <= PRE[t]:
                            nc.vector.max(cands[t][:, sc * 8:(sc + 1) * 8],
                                          bo[:])
                        else:
                            o = 8 * ROUNDS + (sc - PRE[t] - 1) * 8
                            nc.vector.max(tails[t][:, o:o + 8], bo[:])
                        # prefix pass for this tile: extract the chunk-
                        # overflow stat and the prefix top-(8R) while the
                        # PE finishes the remaining chunks
                        if sc == PRE[t]:
                            c8a = smallp.tile([128, 1], f32, tag="c8a",
                                              name=f"c8a_{t}", bufs=NT)
                            cand3 = cands[t][:].rearrange(
                                "p (c e) -> p c e", e=8)
                            nc.vector.tensor_reduce(
                                c8a[:], cand3[:, 0:PRE[t] + 1, 7:8],
                                axis=mybir.AxisListType.XY,
                                op=mybir.AluOpType.max)
                            c8s[t] = c8a
                            emit_half_tau(t)
                        if sc == NSC - 1:
                            # suffix chunk-overflow stat BEFORE the merge
                            # (the merge's match_replace destroys tails)
                            c8 = smallp.tile([128, 1], f32, tag="c8",
                                             name=f"c8_{t}")
                            tail3 = tails[t][:, 8 * ROUNDS:].rearrange(
                                "p (c e) -> p c e", e=8)
                            nc.vector.tensor_reduce(
                                c8[:], tail3[:, :, 7:8],
                                axis=mybir.AxisListType.XY,
                                op=mybir.AluOpType.max)
                            nc.vector.tensor_tensor(
                                c8[:], c8[:], c8s[t][:],
                                op=mybir.AluOpType.max)
                            # tau next, then immediately unblock the PE
                            # with block 0's mask+transpose; flag math
                            # runs off the critical path afterwards
                            topv = emit_tau(t)
                            mask_transpose(t, 0, halves=2)
                            gap = emit_gap(t, topv)
                            # >= : an fp16 tie of the chunk 8th candidate
                            # with tau can hide an unextracted equal value
                            nc.vector.tensor_tensor(
                                flags_sb[:, t:t + 1], c8[:], taus[t][:],
                                op=mybir.AluOpType.is_ge)
                            nc.vector.tensor_tensor(
                                flags_sb[:, t:t + 1], flags_sb[:, t:t + 1],
                                gap[:], op=mybir.AluOpType.add)

            # -------- Phases C+D: mask/transpose + decode ---------------
            with tc.tile_pool(name="decpsum", bufs=NT, space="PSUM") as decpsp, \
                 tc.tile_pool(name="actsc2", bufs=11) as acp2:
                for t in range(NT):
                    load_ac(t, 1, acp2)
                for t in range(NT):
                    load_ac(t, 2, acp2)
                for t in range(NT):
                    mask_transpose(t, 1)

                pss = [decpsp.tile([128, ACT_DIM], f32, tag="dps",
                                   name=f"dps{t}") for t in range(NT)]
                for blk in range(NBLK):
                    # stay >= 2 blocks ahead with mask/transpose, 3 with
                    # spill reloads.  W_dec loads go FIRST in the sync
                    # stream: at the tail they issue ~14us into the block
                    # and their ~11us transfer lands exactly when the
                    # consuming block wants them (a 5-10us PE stall).
                    for fg in (2 * blk + 4, 2 * blk + 5):
                        if fg < NFG:
                            load_wd(fg)
                    if blk + 3 < NBLK:
                        for t in range(NT):
                            load_ac(t, blk + 3, acp2)
                    if blk + 2 < NBLK:
                        for t in range(NT):
                            mask_transpose(t, blk + 2)
                    wd2 = [wds.pop(2 * blk), wds.pop(2 * blk + 1)]
                    for t in range(NT):
                        for g in range(2):
                            wd = wd2[g]
                            for j in range(8):
                                f = blk * 16 + g * 8 + j
                                lhsT = ets[(t, blk)][:, g * 8 + j, :]
                                st = (f == 0)
                                sp = (f == NF - 1)
                                nc.tensor.matmul(
                                    pss[t][:, 0:512], lhsT, wd[:, j, 0:512],
                                    start=st, stop=sp)
                                nc.tensor.matmul(
                                    pss[t][:, 512:ACT_DIM], lhsT,
                                    wd[:, j, 512:ACT_DIM],
                                    start=st, stop=sp)
                        if blk == NBLK - 1:
                            ot = outp.tile([128, ACT_DIM], f32, tag="ot",
                                           name=f"ot{t}")
                            nc.vector.tensor_tensor(
                                ot[:], pss[t][:], bdec_bc[:],
                                op=mybir.AluOpType.add)
                            nc.sync.dma_start(
                                xhat_d.ap()[t * 128:(t + 1) * 128, :], ot[:])
                    for t in range(NT):
                        ets.pop((t, blk))
                nc.sync.dma_start(flags_d.ap(), flags_sb[:])

    nc.compile()
    return nc


def _get_program(k: int, with_benc: bool):
    key = (k, with_benc)
    if key not in _cache:
        _cache[key] = _build(k, with_benc)
    return _cache[key]


def _host_repair(out, rows, x, W_enc, b_enc, W_dec, b_dec, k):
    rows = np.asarray(rows, dtype=np.int64)
    pre = (x[rows] - b_dec) @ W_enc.T + b_enc          # [R, F]
    acts = np.maximum(pre, 0.0)
    idx = np.argsort(-acts, axis=1, kind="stable")[:, :k]
    enc = np.zeros_like(acts)
    np.put_along_axis(enc, idx, np.take_along_axis(acts, idx, 1), 1)
    out[rows] = enc @ W_dec.T + b_dec


def run(inputs, trace=False):
    from concourse.bass_utils import run_bass_kernel_spmd

    x = np.asarray(inputs["x"], dtype=np.float32)
    W_enc = np.asarray(inputs["W_enc"], dtype=np.float32)
    b_enc = np.asarray(inputs["b_enc"], dtype=np.float32)
    W_dec = np.asarray(inputs["W_dec"], dtype=np.float32)
    b_dec = np.asarray(inputs["b_dec"], dtype=np.float32)
    k = int(np.asarray(inputs["k"]))
    assert x.shape == (BATCH, ACT_DIM) and W_enc.shape == (DICT, ACT_DIM)
    assert 1 <= k <= CANDW - 8

    with_benc = bool(np.any(b_enc))
    nc = _get_program(k, with_benc)

    xT = np.ascontiguousarray((x - b_dec).T, dtype=np.float32).astype(np.float16)
    wencT = np.ascontiguousarray(W_enc.T, dtype=np.float32).astype(np.float16)
    wdecT = np.ascontiguousarray(W_dec.T).astype(np.float16)
    wdec_r = np.ascontiguousarray(
        wdecT.reshape(DICT // 1024, 8, 128, ACT_DIM).transpose(0, 2, 1, 3)
        .reshape(DICT // 1024, 128, 8 * ACT_DIM))
    bdec_row = np.ascontiguousarray(b_dec.reshape(1, ACT_DIM))

    in_maps = []
    for c in range(NCORES):
        sl = slice(c * ROWS, (c + 1) * ROWS)
        m = {
            "xr": np.ascontiguousarray(xT[:, sl]),
            "wencR": wencT,
            "wdecT": wdec_r,
            "bdec": bdec_row,
        }
        if with_benc:
            m["benc"] = np.ascontiguousarray(b_enc.reshape(1, DICT))
        in_maps.append(m)

    res = run_bass_kernel_spmd(nc, in_maps, core_ids=list(range(NCORES)),
                               trace=trace)

    out = np.empty((BATCH, ACT_DIM), dtype=np.float32)
    flagged = []
    for c in range(NCORES):
        out[c * ROWS:(c + 1) * ROWS] = res.results[c]["xhat"]
        fl = res.results[c]["flags"]          # [128, NT]
        for t in range(NT):
            for p in np.nonzero(fl[:, t] > 0)[0]:
                flagged.append(c * ROWS + t * 128 + int(p))
    if flagged:
        _host_repair(out, flagged, x, W_enc, b_enc, W_dec, b_dec, k)
    return out, res, flagged


def kernel(**inputs) -> np.ndarray:
    out, _, _ = run(inputs)
    return out


# revision 38
# speedup vs baseline: 1.2805x; 1.0228x over previous
"""TopK sparse autoencoder forward pass on 8 TRN2 NeuronCores.

Data-parallel over the batch (512 rows/core), single-pass fp16 encode:

  A. encode: acts = relu((x - b_dec) @ W_enc.T) as ONE fp16 matmul pass.
     TRN2 fp16 matmul = exact fp16 operand rounding + fp32 accumulation at
     full bf16 rate.  The resulting ~2.8e-4 sigma act noise swaps the
     top-64 selection vs the fp32 reference only on near-tie rows.
     W_enc streams once in fp16 (38 MB/core); acts are rounded to fp16 by
     the relu and spilled to DRAM in fp16 (25 MB/core, half the v1 fp32
     spill -- the dominant DMA saving).  fp16 rounding is monotone, so it
     cannot swap the top-k order; it only creates ties at the k boundary,
     which are flagged and host-repaired.  Per-512-chunk top-8 candidates
     are extracted from the fp16 relu bounce (one MAX8 per chunk).
  B. topk: exact top-(k+8) of each row from the fp16 candidate array via
     max8/match_replace rounds, split into a late mid-encode prefix pass
     (per tile, staggered over sc, covering ~80-92% of chunks) and a very
     short suffix merge at encode end.  tau = k-th value (an exact fp16
     value).  Flags: (a) chunk top-8 overflow (candidate set provably
     incomplete, >= so fp16 ties at the boundary are also caught),
     (b) boundary gap < 2e-4, which for fp16 values means an exact tie
     v_k == v_{k+1} (adjacent fp16 values near 1.5 differ by >= 4.9e-4):
     the mask would then select k+1 values.  Flagged rows are recomputed
     exactly on the host (batched GEMM).
  C. mask: enc = (acts >= tau) * acts from the fp16 spill (exact,
     self-consistent), fp16, DMA-transposed to [F, B] tiles.
  D. decode: x_hat = enc @ W_dec.T + b_dec, all fp16 operands (better
     precision than v1's bf16), block-pipelined over 2048-wide F blocks
     so W_dec streams exactly once.
     Block 0's acts never round-trip DRAM (written SBUF-resident in A).

Schedule: decode is block-pipelined with W_dec groups and the
mask/transpose chain 2 blocks ahead and spill reloads 3 ahead; all DMA
is issued from the sync engine (cross-engine issue measurably loses to
a single well-ordered queue here), with W_dec loads FIRST in each
block's sync stream so their transfer completes a full block before the
PE consumes them.  The masks run in place on the reloaded spill tiles,
giving the mask->transpose ring three blocks of buffer slack.  The
fp16 spill halves the decode reload traffic, which in the fp32 version
exceeded the PE's per-block work and re-triggered the PE half-clock
power state (HAM) every block; block 0's chain is split in halves so
the PE restarts early after the top-k merge.  Measured: ~626-645 us
fast-clock (vs 803 us for the fp32-spill baseline); the device
alternates between clock states worth ~±70 us run-to-run.
"""

import numpy as np

ACT_DIM = 768
DICT = 24576
BATCH = 4096
NCORES = 8
ROWS = BATCH // NCORES          # 512 rows per core
NT = ROWS // 128                # 4 row-tiles per core
CH = 512                        # stage-1 chunk width (= one encode col-chunk)
NCH = DICT // CH                # 48 chunks
CANDW = NCH * 8                 # 384 candidates per row
NEG = -60000.0                  # match_replace sentinel (fp16-representable)
GAP_DELTA = 2.0e-4              # boundary-tie host-repair threshold
NA = ACT_DIM // 128             # 6 K-chunks

_cache = {}


def _build(k: int, with_benc: bool):
    import concourse.bass as bass
    import concourse.mybir as mybir
    from concourse import bacc
    from concourse import tile

    f32 = mybir.dt.float32
    fp16 = mybir.dt.float16
    ROUNDS = (k + 7) // 8 + 1   # one extra round to expose v_{k+1} (gap flag)

    nc = bacc.Bacc("TRN2", target_bir_lowering=False, debug=False,
                   num_devices=NCORES)

    xr_d = nc.dram_tensor("xr", [ACT_DIM, ROWS], fp16, kind="ExternalInput")
    wr_d = nc.dram_tensor("wencR", [ACT_DIM, DICT], fp16, kind="ExternalInput")
    wdecT_d = nc.dram_tensor("wdecT", [DICT // 1024, 128, 8 * ACT_DIM], fp16,
                             kind="ExternalInput")
    bdec_d = nc.dram_tensor("bdec", [1, ACT_DIM], f32, kind="ExternalInput")
    if with_benc:
        benc_d = nc.dram_tensor("benc", [1, DICT], f32, kind="ExternalInput")
    xhat_d = nc.dram_tensor("xhat", [ROWS, ACT_DIM], f32, kind="ExternalOutput")
    flags_d = nc.dram_tensor("flags", [128, NT], f32, kind="ExternalOutput")
    acts_spill = nc.dram_tensor("acts_spill", [NT, 128, DICT], fp16)

    NSC = DICT // 512           # 48 encode column-chunks
    NBLK = DICT // 2048         # 12 C/D blocks
    NF = DICT // 128            # 192 decoder f-chunks
    NFG = DICT // 1024          # 24 wdec groups (2 per block)

    with tile.TileContext(nc) as tc:
        with tc.tile_pool(name="const", bufs=1) as constp, \
             tc.tile_pool(name="cand", bufs=NT) as candp, \
             tc.tile_pool(name="small", bufs=4 * NT + 4) as smallp, \
             tc.tile_pool(name="actsc", bufs=8) as actscp, \
             tc.tile_pool(name="wdec", bufs=4) as wdecp, \
             tc.tile_pool(name="enct", bufs=12) as enctp, \
             tc.tile_pool(name="outsb", bufs=2) as outp:

            bdec_row = constp.tile([1, ACT_DIM], f32)
            nc.sync.dma_start(bdec_row[:], bdec_d.ap())
            bdec_bc = constp.tile([128, ACT_DIM], f32)
            nc.gpsimd.partition_broadcast(bdec_bc[:], bdec_row[:])
            if with_benc:
                benc_row = constp.tile([1, DICT], f32)
                nc.sync.dma_start(benc_row[:], benc_d.ap())

            flags_sb = constp.tile([128, NT], f32)
            cands = [candp.tile([128, CANDW], fp16, tag="cand", name=f"cand{t}")
                     for t in range(NT)]
            taus = [smallp.tile([128, 1], f32, tag="tau", name=f"tau{t}")
                    for t in range(NT)]
            # tails[t]: [0:8R) prefix top-8R, [8R:) suffix chunk candidates,
            # written adjacently so the final merge needs no copies
            PRE = {t: 36 + 2 * t for t in range(NT)}    # prefix pass sc
            sufw = {t: 8 * (NSC - 1 - PRE[t]) for t in range(NT)}
            tails = [candp.tile([128, 8 * ROUNDS + sufw[t]], fp16,
                                tag="tail", name=f"tail{t}", bufs=NT)
                     for t in range(NT)]

            # ---- shared decode-side helpers (pools live at top level) ----
            acs = {}
            ets = {}
            wds = {}

            def load_wd(fg, half=None):
                # half=0/1 loads one 4-chunk half of the group (used during
                # encode so the 1.57MB group DMA never starves the W_enc
                # chunk stream); half=None loads the whole group at once
                if half is None or half == 0:
                    wd = wdecp.tile([128, 8, ACT_DIM], fp16, tag="wd",
                                    name=f"wd{fg}")
                    wds[fg] = wd
                else:
                    wd = wds[fg]
                hs = [0, 1] if half is None else [half]
                for h in hs:
                    nc.sync.dma_start(
                        wd[:, 4 * h:4 * (h + 1), :]
                        .rearrange("p c a -> p (c a)"),
                        wdecT_d.ap()[fg, :, 4 * h * ACT_DIM:
                                     4 * (h + 1) * ACT_DIM])

            def load_ac(t, blk, pool=None):
                ac = (pool or actscp).tile([128, 2048], fp16, tag="ac",
                                           name=f"ac{t}_{blk}")
                nc.sync.dma_start(
                    ac[:],
                    acts_spill.ap()[t, :, blk * 2048:(blk + 1) * 2048])
                acs[(t, blk)] = ac

            def mask_transpose(t, blk, halves=1):
                # mask IN PLACE on the reloaded spill tile: no separate
                # eb pool, so the mask->transpose ring has the full ac
                # pool depth (3 blocks) of slack instead of one block.
                # halves=2 splits the chain so the PE can start consuming
                # the first 8 f-chunks while the rest is still masking
                # (used on the encode->decode critical path for block 0).
                ac = acs.pop((t, blk))
                et = enctp.tile([128, 16, 128], fp16, tag="enct",
                                name=f"et{t}_{blk}")
                hw = 2048 // halves
                for h in range(halves):
                    sl = slice(h * hw, (h + 1) * hw)
                    nc.vector.scalar_tensor_tensor(
                        ac[:, sl], ac[:, sl], taus[t][:, 0:1], ac[:, sl],
                        op0=mybir.AluOpType.is_ge,
                        op1=mybir.AluOpType.mult)
                    nc.sync.dma_start_transpose(
                        et[:, h * (16 // halves):(h + 1) * (16 // halves), :],
                        ac[:, sl])
                ets[(t, blk)] = et

            def topk_rounds(topv, arr):
                for r in range(ROUNDS):
                    nc.vector.max(topv[:, r * 8:(r + 1) * 8], arr)
                    if r < ROUNDS - 1:
                        nc.vector.match_replace(
                            arr, topv[:, r * 8:(r + 1) * 8], arr, NEG)

            # per-tile prefix split of the candidate array: tile t's
            # prefix top-(8R) is computed mid-encode at sc == PRE[t]
            # (staggered so the ~9us DVE chains drain before encode ends).
            # With CH=512 each sc contributes 8 candidates; the prefix
            # covers (PRE[t] + 1) chunks and its top-8R lands in
            # tails[t][:, 0:8R], where the remaining chunks' candidates
            # are appended, so the final merge is one in-place topk.
            def emit_half_tau(t):
                topk_rounds(tails[t][:, 0:8 * ROUNDS],
                            cands[t][:, 0:(PRE[t] + 1) * 8])

            def emit_tau(t):
                # merge: global top-k is within (prefix top-(8R)) U suffix
                topv = smallp.tile([128, 8 * ROUNDS], fp16, tag="topv",
                                   name=f"topv{t}", bufs=NT)
                topk_rounds(topv, tails[t][:])
                nc.vector.tensor_copy(taus[t][:], topv[:, k - 1:k])
                return topv

            def emit_gap(t, topv):
                # tie detector: for fp16 candidate values a boundary gap
                # below GAP_DELTA means v_k == v_{k+1} exactly -- the mask
                # would select k+1 features (or the fp16 rounding collapsed
                # a near-tie the fp32 reference resolves differently)
                gap = smallp.tile([128, 1], f32, tag="gap", name=f"gap{t}",
                                  bufs=NT)
                nc.vector.tensor_tensor(gap[:], topv[:, k - 1:k],
                                        topv[:, k:k + 1],
                                        op=mybir.AluOpType.subtract)
                nc.vector.tensor_scalar(gap[:], gap[:], GAP_DELTA, None,
                                        op0=mybir.AluOpType.is_lt)
                return gap

            # first wdec group early; rest staggered into the encode loop

            # ---------------- Phase A: encode + spill + stage-1 ----------
            with tc.tile_pool(name="xt", bufs=1) as xtp, \
                 tc.tile_pool(name="wenc", bufs=5) as wencp, \
                 tc.tile_pool(name="bounce", bufs=4) as bouncep, \
                 tc.tile_pool(name="encpsum", bufs=6, space="PSUM") as encpsp, \
                 tc.tile_pool(name="bencbc", bufs=2) as bencbcp:

                xr_sb = xtp.tile([128, NA, ROWS], fp16)
                for a in range(NA):
                    nc.sync.dma_start(
                        xr_sb[:, a, :],
                        xr_d.ap()[a * 128:(a + 1) * 128, :])

                c8s = {}
                for sc in range(NSC):
                    if sc in (2, 10, 18, 26):
                        load_wd(sc // 8, half=0)
                    elif sc in (6, 14, 22, 30):
                        load_wd((sc - 4) // 8, half=1)
                    wch = wencp.tile([128, NA, 512], fp16, tag="w",
                                     name=f"w{sc}")
                    nc.sync.dma_start(
                        wch[:],
                        wr_d.ap()[:, sc * 512:(sc + 1) * 512]
                        .rearrange("(a p) c -> p a c", p=128))
                    if with_benc:
                        bb = bencbcp.tile([128, 512], f32, tag="bb")
                        nc.gpsimd.partition_broadcast(
                            bb[:], benc_row[0:1, sc * 512:(sc + 1) * 512])
                    for t in range(NT):
                        ps = encpsp.tile([128, 512], f32, tag="eps")
                        rt = slice(t * 128, (t + 1) * 128)
                        for a in range(NA):
                            nc.tensor.matmul(
                                ps[:], xr_sb[:, a, rt], wch[:, a, :],
                                start=(a == 0), stop=(a == NA - 1))
                        if sc < 8:
                            rb = sc // 4
                            if (t, rb) not in acs:
                                acs[(t, rb)] = actscp.tile(
                                    [128, 2048], fp16, tag="ac",
                                    name=f"ac{t}_{rb}")
                            bo = acs[(t, rb)][:, (sc % 4) * 512:
                                              (sc % 4 + 1) * 512]
                        else:
                            bo = bouncep.tile([128, 512], fp16, tag="bo")
                        if with_benc:
                            nc.vector.tensor_tensor(bo[:], ps[:], bb[:],
                                                    op=mybir.AluOpType.add)
                            nc.scalar.activation(
                                bo[:], bo[:], mybir.ActivationFunctionType.Relu)
                        else:
                            nc.scalar.activation(
                                bo[:], ps[:], mybir.ActivationFunctionType.Relu)
                        if sc >= 8:
                            nc.sync.dma_start(
                                acts_spill.ap()[t, :, sc * 512:(sc + 1) * 512],
                                bo[:])
                        if sc <= PRE[t]:
                            nc.vector.max(cands[t][:, sc * 8:(sc + 1) * 8],
                                          bo[:])
                        else:
                            o = 8 * ROUNDS + (sc - PRE[t] - 1) * 8
                            nc.vector.max(tails[t][:, o:o + 8], bo[:])
                        # prefix pass for this tile: extract the chunk-
                        # overflow stat and the prefix top-(8R) while the
                        # PE finishes the remaining chunks
                        if sc == PRE[t]:
                            c8a = smallp.tile([128, 1], f32, tag="c8a",
                                              name=f"c8a_{t}", bufs=NT)
                            cand3 = cands[t][:].rearrange(
                                "p (c e) -> p c e", e=8)
                            nc.vector.tensor_reduce(
                                c8a[:], cand3[:, 0:PRE[t] + 1, 7:8],
                                axis=mybir.AxisListType.XY,
                                op=mybir.AluOpType.max)
                            c8s[t] = c8a
                            emit_half_tau(t)
                        if sc == NSC - 1:
                            # suffix chunk-overflow stat BEFORE the merge
                            # (the merge's match_replace destroys tails)
                            c8 = smallp.tile([128, 1], f32, tag="c8",
                                             name=f"c8_{t}")
                            tail3 = tails[t][:, 8 * ROUNDS:].rearrange(
                                "p (c e) -> p c e", e=8)
                            nc.vector.tensor_reduce(
                                c8[:], tail3[:, :, 7:8],
                                axis=mybir.AxisListType.XY,
                                op=mybir.AluOpType.max)
                            nc.vector.tensor_tensor(
                                c8[:], c8[:], c8s[t][:],
                                op=mybir.AluOpType.max)
                            # tau next, then immediately unblock the PE
                            # with block 0's mask+transpose; flag math
                            # runs off the critical path afterwards
                            topv = emit_tau(t)
                            mask_transpose(t, 0, halves=2)
                            gap = emit_gap(t, topv)
                            # >= : an fp16 tie of the chunk 8th candidate
                            # with tau can hide an unextracted equal value
                            nc.vector.tensor_tensor(
                                flags_sb[:, t:t + 1], c8[:], taus[t][:],
                                op=mybir.AluOpType.is_ge)
                            nc.vector.tensor_tensor(
                                flags_sb[:, t:t + 1], flags_sb[:, t:t + 1],
                                gap[:], op=mybir.AluOpType.add)

            # -------- Phases C+D: mask/transpose + decode ---------------
            with tc.tile_pool(name="decpsum", bufs=NT, space="PSUM") as decpsp, \
                 tc.tile_pool(name="actsc2", bufs=11) as acp2:
                for t in range(NT):
                    load_ac(t, 2, acp2)
                for t in range(NT):
                    load_ac(t, 3, acp2)
                for t in range(NT):
                    mask_transpose(t, 1)

                pss = [decpsp.tile([128, ACT_DIM], f32, tag="dps",
                                   name=f"dps{t}") for t in range(NT)]
                for blk in range(NBLK):
                    # stay >= 2 blocks ahead with mask/transpose, 3 with
                    # spill reloads.  W_dec loads go FIRST in the sync
                    # stream: at the tail they issue ~14us into the block
                    # and their ~11us transfer lands exactly when the
                    # consuming block wants them (a 5-10us PE stall).
                    for fg in (2 * blk + 4, 2 * blk + 5):
                        if fg < NFG:
                            load_wd(fg)
                    if blk >= 1 and blk + 3 < NBLK:
                        for t in range(NT):
                            load_ac(t, blk + 3, acp2)
                    if blk + 2 < NBLK:
                        for t in range(NT):
                            mask_transpose(t, blk + 2)
                    wd2 = [wds.pop(2 * blk), wds.pop(2 * blk + 1)]
                    for t in range(NT):
                        for g in range(2):
                            wd = wd2[g]
                            for j in range(8):
                                f = blk * 16 + g * 8 + j
                                lhsT = ets[(t, blk)][:, g * 8 + j, :]
                                st = (f == 0)
                                sp = (f == NF - 1)
                                nc.tensor.matmul(
                                    pss[t][:, 0:512], lhsT, wd[:, j, 0:512],
                                    start=st, stop=sp)
                                nc.tensor.matmul(
                                    pss[t][:, 512:ACT_DIM], lhsT,
                                    wd[:, j, 512:ACT_DIM],
                                    start=st, stop=sp)
                        if blk == NBLK - 1:
                            ot = outp.tile([128, ACT_DIM], f32, tag="ot",
                                           name=f"ot{t}")
                            nc.vector.tensor_tensor(
                                ot[:], pss[t][:], bdec_bc[:],
                                op=mybir.AluOpType.add)
                            nc.sync.dma_start(
                                xhat_d.ap()[t * 128:(t + 1) * 128, :], ot[:])
                    for t in range(NT):
                        ets.pop((t, blk))
                nc.sync.dma_start(flags_d.ap(), flags_sb[:])

    nc.compile()
    return nc


def _get_program(k: int, with_benc: bool):
    key = (k, with_benc)
    if key not in _cache:
        _cache[key] = _build(k, with_benc)
    return _cache[key]


def _host_repair(out, rows, x, W_enc, b_enc, W_dec, b_dec, k):
    rows = np.asarray(rows, dtype=np.int64)
    pre = (x[rows] - b_dec) @ W_enc.T + b_enc          # [R, F]
    acts = np.maximum(pre, 0.0)
    idx = np.argsort(-acts, axis=1, kind="stable")[:, :k]
    enc = np.zeros_like(acts)
    np.put_along_axis(enc, idx, np.take_along_axis(acts, idx, 1), 1)
    out[rows] = enc @ W_dec.T + b_dec


def run(inputs, trace=False):
    from concourse.bass_utils import run_bass_kernel_spmd

    x = np.asarray(inputs["x"], dtype=np.float32)
    W_enc = np.asarray(inputs["W_enc"], dtype=np.float32)
    b_enc = np.asarray(inputs["b_enc"], dtype=np.float32)
    W_dec = np.asarray(inputs["W_dec"], dtype=np.float32)
    b_dec = np.asarray(inputs["b_dec"], dtype=np.float32)
    k = int(np.asarray(inputs["k"]))
    assert x.shape == (BATCH, ACT_DIM) and W_enc.shape == (DICT, ACT_DIM)
    assert 1 <= k <= CANDW - 8

    with_benc = bool(np.any(b_enc))
    nc = _get_program(k, with_benc)

    xT = np.ascontiguousarray((x - b_dec).T, dtype=np.float32).astype(np.float16)
    wencT = np.ascontiguousarray(W_enc.T, dtype=np.float32).astype(np.float16)
    wdecT = np.ascontiguousarray(W_dec.T).astype(np.float16)
    wdec_r = np.ascontiguousarray(
        wdecT.reshape(DICT // 1024, 8, 128, ACT_DIM).transpose(0, 2, 1, 3)
        .reshape(DICT // 1024, 128, 8 * ACT_DIM))
    bdec_row = np.ascontiguousarray(b_dec.reshape(1, ACT_DIM))

    in_maps = []
    for c in range(NCORES):
        sl = slice(c * ROWS, (c + 1) * ROWS)
        m = {
            "xr": np.ascontiguousarray(xT[:, sl]),
            "wencR": wencT,
            "wdecT": wdec_r,
            "bdec": bdec_row,
        }
        if with_benc:
            m["benc"] = np.ascontiguousarray(b_enc.reshape(1, DICT))
        in_maps.append(m)

    res = run_bass_kernel_spmd(nc, in_maps, core_ids=list(range(NCORES)),
                               trace=trace)

    out = np.empty((BATCH, ACT_DIM), dtype=np.float32)
    flagged = []
    for c in range(NCORES):
        out[c * ROWS:(c + 1) * ROWS] = res.results[c]["xhat"]
        fl = res.results[c]["flags"]          # [128, NT]
        for t in range(NT):
            for p in np.nonzero(fl[:, t] > 0)[0]:
                flagged.append(c * ROWS + t * 128 + int(p))
    if flagged:
        _host_repair(out, flagged, x, W_enc, b_enc, W_dec, b_dec, k)
    return out, res, flagged


def kernel(**inputs) -> np.ndarray:
    out, _, _ = run(inputs)
    return out
